# revision 23
# baseline (speedup 1.0000x reference)
"""Trainium2 Bass kernel for nn_CrossAttention (sigmoid cross-attention).

Full-input contract: kernel(**inputs) takes the complete tensors and returns
(out, attn) exactly like the reference. Internally shards across 8 NeuronCores:
data-parallel over batch (B=2) x tensor-parallel over head groups (16 heads ->
4 per core). Each core:
  - transposes its x/y slices on the PE array,
  - computes qT/kT/v for its 4 heads (f32r matmuls, full PE rate),
  - computes scores in BOTH orientations: [n,m] tiles feed the attn HBM output,
    [m,n] tiles feed the attn@v matmul (avoids transposing the 16MB attn
    matrix per head),
  - computes its partial out-projection (rows of Wproj for its heads).
Host glue sums the 4 per-batch proj partials (+ bias) and concatenates attn.
"""

import math
import sys

import numpy as np

for _p in ("/opt/trn_rl_repo", "/root/.axon_site/_ro/trn_rl_repo"):
    if _p not in sys.path:
        sys.path.insert(0, _p)

import concourse.bass as bass
import concourse.bacc as bacc_mod
import concourse.mybir as mybir
from concourse.bass_utils import run_bass_kernel_spmd
from concourse.masks import make_identity
from concourse.tile import TileContext

B, N, NY = 2, 2048, 2048
C = 1024
H = 16
HD = C // H            # 64
SCALE = HD ** -0.5
BIAS = -math.log(NY)
NCORES = 8
HL = H // (NCORES // B)   # 4 local heads per core
DL = HL * HD              # 256 local head-dim columns
F32 = mybir.dt.float32
F32R = mybir.dt.float32r

EB = C // 128    # 8 e-blocks (contraction over model dim)
NB = N // 128    # 16
NB4 = N // 512   # 4
MB = NY // 128   # 16
MB4 = NY // 512  # 4




def build_program():
    nc = bacc_mod.Bacc()
    xb = nc.dram_tensor("xb", [N, C], F32, kind="ExternalInput")
    yb = nc.dram_tensor("yb", [NY, C], F32, kind="ExternalInput")
    wq = nc.dram_tensor("wq", [C, DL], F32R, kind="ExternalInput")
    wk = nc.dram_tensor("wk", [C, DL], F32R, kind="ExternalInput")
    wv = nc.dram_tensor("wv", [C, DL], F32R, kind="ExternalInput")
    wp = nc.dram_tensor("wp", [DL, C], F32R, kind="ExternalInput")
    attn_o = nc.dram_tensor("attn_o", [HL, NY, N], F32R, kind="ExternalOutput")
    out_o = nc.dram_tensor("out_o", [N, C], F32, kind="ExternalOutput")

    with TileContext(nc) as tc:
        from contextlib import ExitStack

        with ExitStack() as es:
            const_pool = es.enter_context(tc.tile_pool(name="const", bufs=1))
            ident = const_pool.tile([128, 128], F32)
            make_identity(nc, ident[:])
            bias_sb = const_pool.tile([128, 1], F32, tag="bias")
            nc.vector.memset(bias_sb[:], BIAS)

            w_pool = es.enter_context(tc.tile_pool(name="weights", bufs=1))
            wq_sb = w_pool.tile([128, EB * DL], F32R, tag="wq")
            wk_sb = w_pool.tile([128, EB * DL], F32R, tag="wk")
            wv_sb = w_pool.tile([128, EB * DL], F32R, tag="wv")
            wp_sb = [w_pool.tile([128, C], F32R, tag=f"wp{k}", name=f"wp{k}") for k in range(2)]
            for wsrc, wdst in ((wk, wk_sb), (wv, wv_sb)):
                nc.sync.dma_start(
                    out=wdst[:].rearrange("p (e d) -> p e d", e=EB),
                    in_=wsrc.rearrange("(e p) d -> p e d", p=128),
                )

            per_pool = es.enter_context(tc.tile_pool(name="persist", bufs=1))
            qT_sb = [per_pool.tile([128, N], F32R, tag=f"qT{d}", name=f"qT{d}") for d in range(2)]
            kT_sb = [per_pool.tile([128, NY], F32R, tag=f"kT{d}", name=f"kT{d}") for d in range(2)]
            v_sb = per_pool.tile([128, MB * DL], F32R, tag="v")
            o2T_sb = [per_pool.tile([128, N], F32R, tag=f"o2T{d}", name=f"o2T{d}") for d in range(2)]

            CH = 512  # row-chunk for streamed transpose+project
            NCH = N // CH   # 4

            def transpose_chunk(src_dram, ch, dst_sb, xin_pool, tr_psum):
                # rows [ch*CH, (ch+1)*CH) of src [*, C] -> dst_sb [128, EB*CH]
                dst3 = dst_sb[:].rearrange("p (e n) -> p e n", e=EB)
                for nb in range(CH // 128):
                    xin = xin_pool.tile([128, C], F32, tag="xin")
                    r0 = ch * CH + nb * 128
                    nc.sync.dma_start(out=xin[:, :], in_=src_dram[r0:r0 + 128, :])
                    for ebp in range(2):
                        ptr = tr_psum.tile([128, 512], F32, tag="trp")
                        for j in range(4):
                            eb = ebp * 4 + j
                            nc.tensor.transpose(
                                ptr[:, j * 128:(j + 1) * 128],
                                xin[:, eb * 128:(eb + 1) * 128],
                                ident[:],
                            )
                        src3 = ptr[:].rearrange("p (j n) -> p j n", j=4)
                        nc.vector.tensor_copy(
                            dst3[:, ebp * 4:(ebp + 1) * 4,
                                 nb * 128:(nb + 1) * 128],
                            src3,
                        )

            # ---- phases A+B+C: streamed transpose/project with head-0
            # attention interleaved into the y-chunk pipeline ----
            sig = mybir.ActivationFunctionType.Sigmoid
            with tc.tile_pool(name="tchunk", bufs=3) as tch_pool, \
                 tc.tile_pool(name="xin", bufs=4) as xin_pool, \
                 tc.tile_pool(name="trps", bufs=2, space="PSUM") as tr_psum, \
                 tc.tile_pool(name="prpsum", bufs=2, space="PSUM") as pr_psum, \
                 tc.tile_pool(name="atT", bufs=4) as atT_pool, \
                 tc.tile_pool(name="tpsum", bufs=2, space="PSUM") as t_psum, \
                 tc.tile_pool(name="opsum", bufs=1, space="PSUM") as o_psum:

                def head_pass(hh, n4):
                    # one n4 column-pass of a head: accumulate attn @ v over mb
                    d = hh // 2
                    po = (hh % 2) * HD
                    qTh = qT_sb[d]
                    kTh = kT_sb[d]
                    pso = o_psum.tile([64, 512], F32, tag="op", name="pso")
                    for mb in range(MB):
                        atT = atT_pool.tile([128, 512], F32R, tag="atT", name="atT")
                        pst = t_psum.tile([128, 512], F32, tag="tp", name="pst")
                        nc.tensor.matmul(
                            pst[:, :],
                            (kTh[po:po + HD, mb * 128:(mb + 1) * 128]),
                            (qTh[po:po + HD, n4 * 512:(n4 + 1) * 512]),
                            start=True, stop=True, skip_group_check=True,
                        )
                        nc.scalar.activation(
                            atT[:, :], pst[:, :],
                            sig, bias=bias_sb[:, :], scale=SCALE)
                        nc.tensor.matmul(
                            pso[:, :],
                            (v_sb[:, mb * DL + hh * HD: mb * DL + (hh + 1) * HD]),
                            (atT[:, :]),
                            start=(mb == 0), stop=(mb == MB - 1),
                            skip_group_check=True,
                        )
                        nc.sync.dma_start(
                            out=attn_o[hh, mb * 128:(mb + 1) * 128,
                                       n4 * 512:(n4 + 1) * 512],
                            in_=atT[:, :])
                    nc.vector.tensor_copy(
                        o2T_sb[d][po:po + HD, n4 * 512:(n4 + 1) * 512],
                        pso[:, :])

                for ch in range(NCH):
                    yTc = tch_pool.tile([128, EB * CH], F32R, tag="tc", name="yTc")
                    transpose_chunk(yb, ch, yTc, xin_pool, tr_psum)
                    for d in range(2):
                        ps = pr_psum.tile([128, 512], F32, tag="qp", name="psk")
                        for eb in range(EB):
                            nc.tensor.matmul(
                                ps[:, :],
                                (wk_sb[:, eb * DL + d * 128: eb * DL + (d + 1) * 128]),
                                (yTc[:, eb * CH:(eb + 1) * CH]),
                                start=(eb == 0), stop=(eb == EB - 1),
                            )
                        nc.vector.tensor_copy(
                            kT_sb[d][:, ch * CH:(ch + 1) * CH], ps[:, :])
                    for mo in range(CH // 128):
                        mb = ch * (CH // 128) + mo
                        ps = pr_psum.tile([128, DL], F32, tag="qp", name="psv")
                        for eb in range(EB):
                            nc.tensor.matmul(
                                ps[:, :],
                                (yTc[:, eb * CH + mo * 128: eb * CH + (mo + 1) * 128]),
                                (wv_sb[:, eb * DL:(eb + 1) * DL]),
                                start=(eb == 0), stop=(eb == EB - 1),
                            )
                        nc.vector.tensor_copy(
                            v_sb[:, mb * DL:(mb + 1) * DL], ps[:, :])

                for ch in range(NCH):
                    if ch == 0:
                        nc.sync.dma_start(
                            out=wq_sb[:].rearrange("p (e d) -> p e d", e=EB),
                            in_=wq.rearrange("(e p) d -> p e d", p=128),
                        )
                    xTc = tch_pool.tile([128, EB * CH], F32R, tag="tc", name="xTc")
                    transpose_chunk(xb, ch, xTc, xin_pool, tr_psum)
                    for d in range(2):
                        ps = pr_psum.tile([128, 512], F32, tag="qp", name="psq")
                        for eb in range(EB):
                            nc.tensor.matmul(
                                ps[:, :],
                                (wq_sb[:, eb * DL + d * 128: eb * DL + (d + 1) * 128]),
                                (xTc[:, eb * CH:(eb + 1) * CH]),
                                start=(eb == 0), stop=(eb == EB - 1),
                            )
                        nc.vector.tensor_copy(
                            qT_sb[d][:, ch * CH:(ch + 1) * CH], ps[:, :])

                    head_pass(0, ch)
                for k in range(2):
                    nc.sync.dma_start(out=wp_sb[k][:, :],
                                      in_=wp[k * 128:(k + 1) * 128, :])

                for hh in range(1, HL):
                    for n4 in range(NB4):
                        head_pass(hh, n4)

            # ---- phase D: partial out projection ----
            with tc.tile_pool(name="oproj", bufs=3) as op_pool, \
                 tc.tile_pool(name="ppsum", bufs=2, space="PSUM") as p_psum:
                for nb in range(NB):
                    op = op_pool.tile([128, C], F32, tag="op")
                    for cb in range(2):
                        ps = p_psum.tile([128, 512], F32, tag="pp")
                        for kb in range(2):
                            nc.tensor.matmul(
                                ps[:, :],
                                (o2T_sb[kb][:, nb * 128:(nb + 1) * 128]),
                                (wp_sb[kb][:, cb * 512:(cb + 1) * 512]),
                                start=(kb == 0), stop=(kb == 1),
                            )
                        nc.vector.tensor_copy(op[:, cb * 512:(cb + 1) * 512],
                                              ps[:, :])
                    nc.sync.dma_start(out=out_o[nb * 128:(nb + 1) * 128, :],
                                      in_=op[:, :])
    nc.finalize()
    return nc


_NC_CACHE = None


def _get_nc():
    global _NC_CACHE
    if _NC_CACHE is None:
        _NC_CACHE = build_program()
    return _NC_CACHE


def run(x, y, Wq, Wkv, Wproj, bproj, **spmd_kwargs):
    x = np.asarray(x, dtype=np.float32)
    y = np.asarray(y, dtype=np.float32)
    Wq = np.asarray(Wq, dtype=np.float32)
    Wkv = np.asarray(Wkv, dtype=np.float32)
    Wproj = np.asarray(Wproj, dtype=np.float32)
    bproj = np.asarray(bproj, dtype=np.float32)

    nc = _get_nc()
    in_maps = []
    for c in range(NCORES):
        b, hg = divmod(c, NCORES // B)
        cs = slice(hg * DL, (hg + 1) * DL)
        in_maps.append({
            "xb": np.ascontiguousarray(x[b]),
            "yb": np.ascontiguousarray(y[b]),
            "wq": np.ascontiguousarray(Wq[:, cs]),
            "wk": np.ascontiguousarray(Wkv[:, :C][:, cs]),
            "wv": np.ascontiguousarray(Wkv[:, C:][:, cs]),
            "wp": np.ascontiguousarray(Wproj[cs, :]),
        })
    bkr = run_bass_kernel_spmd(nc, in_maps, list(range(NCORES)), **spmd_kwargs)
    res = bkr.results

    attn = np.empty((B, H, N, NY), dtype=np.float32)
    out = np.zeros((B, N, C), dtype=np.float32)
    for c in range(NCORES):
        b, hg = divmod(c, NCORES // B)
        attn[b, hg * HL:(hg + 1) * HL] = res[c]["attn_o"].transpose(0, 2, 1)
        out[b] += res[c]["out_o"]
    out += bproj[None, None, :]
    return out, attn, bkr


def kernel(x, y, Wq, Wkv, Wproj, bproj):
    out, attn, _ = run(x, y, Wq, Wkv, Wproj, bproj)
    return out, attn


# revision 28
# speedup vs baseline: 1.0292x; 1.0292x over previous
"""Trainium2 Bass kernel for nn_CrossAttention (sigmoid cross-attention).

Full-input contract: kernel(**inputs) takes the complete tensors and returns
(out, attn) exactly like the reference. Internally shards across 8 NeuronCores:
data-parallel over batch (B=2) x tensor-parallel over head groups (16 heads ->
4 per core). Each core:
  - transposes its x/y slices on the PE array,
  - computes qT/kT/v for its 4 heads (f32r matmuls, full PE rate),
  - computes scores in BOTH orientations: [n,m] tiles feed the attn HBM output,
    [m,n] tiles feed the attn@v matmul (avoids transposing the 16MB attn
    matrix per head),
  - computes its partial out-projection (rows of Wproj for its heads).
Host glue sums the 4 per-batch proj partials (+ bias) and concatenates attn.
"""

import math
import sys

import numpy as np

for _p in ("/opt/trn_rl_repo", "/root/.axon_site/_ro/trn_rl_repo"):
    if _p not in sys.path:
        sys.path.insert(0, _p)

import concourse.bass as bass
import concourse.bacc as bacc_mod
import concourse.mybir as mybir
from concourse.bass_utils import run_bass_kernel_spmd
from concourse.masks import make_identity
from concourse.tile import TileContext

B, N, NY = 2, 2048, 2048
C = 1024
H = 16
HD = C // H            # 64
SCALE = HD ** -0.5
BIAS = -math.log(NY)
NCORES = 8
HL = H // (NCORES // B)   # 4 local heads per core
DL = HL * HD              # 256 local head-dim columns
F32 = mybir.dt.float32
F32R = mybir.dt.float32r

EB = C // 128    # 8 e-blocks (contraction over model dim)
NB = N // 128    # 16
NB4 = N // 512   # 4
MB = NY // 128   # 16
MB4 = NY // 512  # 4




def build_program():
    nc = bacc_mod.Bacc()
    xb = nc.dram_tensor("xb", [N, C], F32, kind="ExternalInput")
    yb = nc.dram_tensor("yb", [NY, C], F32, kind="ExternalInput")
    wq = nc.dram_tensor("wq", [C, DL], F32R, kind="ExternalInput")
    wk = nc.dram_tensor("wk", [C, DL], F32R, kind="ExternalInput")
    wv = nc.dram_tensor("wv", [C, DL], F32R, kind="ExternalInput")
    wp = nc.dram_tensor("wp", [DL, C], F32R, kind="ExternalInput")
    attn_o = nc.dram_tensor("attn_o", [HL, NY, N], F32R, kind="ExternalOutput")
    out_o = nc.dram_tensor("out_o", [N, C], F32, kind="ExternalOutput")

    with TileContext(nc) as tc:
        from contextlib import ExitStack

        with ExitStack() as es:
            const_pool = es.enter_context(tc.tile_pool(name="const", bufs=1))
            ident = const_pool.tile([128, 128], F32)
            make_identity(nc, ident[:])
            bias_sb = const_pool.tile([128, 1], F32, tag="bias")
            nc.vector.memset(bias_sb[:], BIAS)

            w_pool = es.enter_context(tc.tile_pool(name="weights", bufs=1))
            wq_sb = w_pool.tile([128, EB * DL], F32R, tag="wq")
            wk_sb = w_pool.tile([128, EB * DL], F32R, tag="wk")
            wv_sb = w_pool.tile([128, EB * DL], F32R, tag="wv")
            wp_sb = [w_pool.tile([128, C], F32R, tag=f"wp{k}", name=f"wp{k}") for k in range(2)]
            for wsrc, wdst in ((wk, wk_sb), (wv, wv_sb)):
                nc.sync.dma_start(
                    out=wdst[:].rearrange("p (e d) -> p e d", e=EB),
                    in_=wsrc.rearrange("(e p) d -> p e d", p=128),
                )

            per_pool = es.enter_context(tc.tile_pool(name="persist", bufs=1))
            qT_sb = [per_pool.tile([128, N], F32R, tag=f"qT{d}", name=f"qT{d}") for d in range(2)]
            kT_sb = [per_pool.tile([128, NY], F32R, tag=f"kT{d}", name=f"kT{d}") for d in range(2)]
            v_sb = per_pool.tile([128, MB * DL], F32R, tag="v")
            o2T_sb = [per_pool.tile([128, N], F32R, tag=f"o2T{d}", name=f"o2T{d}") for d in range(2)]

            CH = 512  # row-chunk for streamed transpose+project
            NCH = N // CH   # 4

            def transpose_chunk(src_dram, ch, dst_sb, xin_pool, tr_psum):
                # rows [ch*CH, (ch+1)*CH) of src [*, C] -> dst_sb [128, EB*CH]
                dst3 = dst_sb[:].rearrange("p (e n) -> p e n", e=EB)
                for nb in range(CH // 128):
                    xin = xin_pool.tile([128, C], F32, tag="xin")
                    r0 = ch * CH + nb * 128
                    nc.sync.dma_start(out=xin[:, :], in_=src_dram[r0:r0 + 128, :])
                    for ebp in range(2):
                        ptr = tr_psum.tile([128, 512], F32, tag="trp")
                        for j in range(4):
                            eb = ebp * 4 + j
                            nc.tensor.transpose(
                                ptr[:, j * 128:(j + 1) * 128],
                                xin[:, eb * 128:(eb + 1) * 128],
                                ident[:],
                            )
                        src3 = ptr[:].rearrange("p (j n) -> p j n", j=4)
                        nc.vector.tensor_copy(
                            dst3[:, ebp * 4:(ebp + 1) * 4,
                                 nb * 128:(nb + 1) * 128],
                            src3,
                        )

            # ---- phases A+B+C: streamed transpose/project with head-0
            # attention interleaved into the y-chunk pipeline ----
            sig = mybir.ActivationFunctionType.Sigmoid
            with tc.tile_pool(name="tchunk", bufs=3) as tch_pool, \
                 tc.tile_pool(name="xin", bufs=11) as xin_pool, \
                 tc.tile_pool(name="trps", bufs=2, space="PSUM") as tr_psum, \
                 tc.tile_pool(name="prpsum", bufs=2, space="PSUM") as pr_psum, \
                 tc.tile_pool(name="atT", bufs=4) as atT_pool, \
                 tc.tile_pool(name="tpsum", bufs=2, space="PSUM") as t_psum, \
                 tc.tile_pool(name="opsum", bufs=1, space="PSUM") as o_psum:

                def head_pass(hh, n4):
                    # one n4 column-pass of a head: accumulate attn @ v over mb
                    d = hh // 2
                    po = (hh % 2) * HD
                    qTh = qT_sb[d]
                    kTh = kT_sb[d]
                    pso = o_psum.tile([64, 512], F32, tag="op", name="pso")
                    for mb in range(MB):
                        atT = atT_pool.tile([128, 512], F32R, tag="atT", name="atT")
                        pst = t_psum.tile([128, 512], F32, tag="tp", name="pst")
                        nc.tensor.matmul(
                            pst[:, :],
                            (kTh[po:po + HD, mb * 128:(mb + 1) * 128]),
                            (qTh[po:po + HD, n4 * 512:(n4 + 1) * 512]),
                            start=True, stop=True, skip_group_check=True,
                        )
                        nc.scalar.activation(
                            atT[:, :], pst[:, :],
                            sig, bias=bias_sb[:, :], scale=SCALE)
                        nc.tensor.matmul(
                            pso[:, :],
                            (v_sb[:, mb * DL + hh * HD: mb * DL + (hh + 1) * HD]),
                            (atT[:, :]),
                            start=(mb == 0), stop=(mb == MB - 1),
                            skip_group_check=True,
                        )
                        nc.sync.dma_start(
                            out=attn_o[hh, mb * 128:(mb + 1) * 128,
                                       n4 * 512:(n4 + 1) * 512],
                            in_=atT[:, :])
                    nc.vector.tensor_copy(
                        o2T_sb[d][po:po + HD, n4 * 512:(n4 + 1) * 512],
                        pso[:, :])

                for ch in range(NCH):
                    yTc = tch_pool.tile([128, EB * CH], F32R, tag="tc", name="yTc")
                    transpose_chunk(yb, ch, yTc, xin_pool, tr_psum)
                    for d in range(2):
                        ps = pr_psum.tile([128, 512], F32, tag="qp", name="psk")
                        for eb in range(EB):
                            nc.tensor.matmul(
                                ps[:, :],
                                (wk_sb[:, eb * DL + d * 128: eb * DL + (d + 1) * 128]),
                                (yTc[:, eb * CH:(eb + 1) * CH]),
                                start=(eb == 0), stop=(eb == EB - 1),
                            )
                        nc.vector.tensor_copy(
                            kT_sb[d][:, ch * CH:(ch + 1) * CH], ps[:, :])
                    for mo in range(CH // 128):
                        mb = ch * (CH // 128) + mo
                        ps = pr_psum.tile([128, DL], F32, tag="qp", name="psv")
                        for eb in range(EB):
                            nc.tensor.matmul(
                                ps[:, :],
                                (yTc[:, eb * CH + mo * 128: eb * CH + (mo + 1) * 128]),
                                (wv_sb[:, eb * DL:(eb + 1) * DL]),
                                start=(eb == 0), stop=(eb == EB - 1),
                            )
                        nc.vector.tensor_copy(
                            v_sb[:, mb * DL:(mb + 1) * DL], ps[:, :])

                for ch in range(NCH):
                    if ch == 0:
                        nc.sync.dma_start(
                            out=wq_sb[:].rearrange("p (e d) -> p e d", e=EB),
                            in_=wq.rearrange("(e p) d -> p e d", p=128),
                        )
                    xTc = tch_pool.tile([128, EB * CH], F32R, tag="tc", name="xTc")
                    transpose_chunk(xb, ch, xTc, xin_pool, tr_psum)
                    for d in range(2):
                        ps = pr_psum.tile([128, 512], F32, tag="qp", name="psq")
                        for eb in range(EB):
                            nc.tensor.matmul(
                                ps[:, :],
                                (wq_sb[:, eb * DL + d * 128: eb * DL + (d + 1) * 128]),
                                (xTc[:, eb * CH:(eb + 1) * CH]),
                                start=(eb == 0), stop=(eb == EB - 1),
                            )
                        nc.vector.tensor_copy(
                            qT_sb[d][:, ch * CH:(ch + 1) * CH], ps[:, :])

                    head_pass(0, ch)
                for k in range(2):
                    nc.sync.dma_start(out=wp_sb[k][:, :],
                                      in_=wp[k * 128:(k + 1) * 128, :])

                for hh in range(1, HL):
                    for n4 in range(NB4):
                        head_pass(hh, n4)

            # ---- phase D: partial out projection ----
            with tc.tile_pool(name="oproj", bufs=3) as op_pool, \
                 tc.tile_pool(name="ppsum", bufs=2, space="PSUM") as p_psum:
                for nb in range(NB):
                    op = op_pool.tile([128, C], F32, tag="op")
                    for cb in range(2):
                        ps = p_psum.tile([128, 512], F32, tag="pp")
                        for kb in range(2):
                            nc.tensor.matmul(
                                ps[:, :],
                                (o2T_sb[kb][:, nb * 128:(nb + 1) * 128]),
                                (wp_sb[kb][:, cb * 512:(cb + 1) * 512]),
                                start=(kb == 0), stop=(kb == 1),
                            )
                        nc.vector.tensor_copy(op[:, cb * 512:(cb + 1) * 512],
                                              ps[:, :])
                    nc.sync.dma_start(out=out_o[nb * 128:(nb + 1) * 128, :],
                                      in_=op[:, :])
    nc.finalize()
    return nc


_NC_CACHE = None


def _get_nc():
    global _NC_CACHE
    if _NC_CACHE is None:
        _NC_CACHE = build_program()
    return _NC_CACHE


def run(x, y, Wq, Wkv, Wproj, bproj, **spmd_kwargs):
    x = np.asarray(x, dtype=np.float32)
    y = np.asarray(y, dtype=np.float32)
    Wq = np.asarray(Wq, dtype=np.float32)
    Wkv = np.asarray(Wkv, dtype=np.float32)
    Wproj = np.asarray(Wproj, dtype=np.float32)
    bproj = np.asarray(bproj, dtype=np.float32)

    nc = _get_nc()
    in_maps = []
    for c in range(NCORES):
        b, hg = divmod(c, NCORES // B)
        cs = slice(hg * DL, (hg + 1) * DL)
        in_maps.append({
            "xb": np.ascontiguousarray(x[b]),
            "yb": np.ascontiguousarray(y[b]),
            "wq": np.ascontiguousarray(Wq[:, cs]),
            "wk": np.ascontiguousarray(Wkv[:, :C][:, cs]),
            "wv": np.ascontiguousarray(Wkv[:, C:][:, cs]),
            "wp": np.ascontiguousarray(Wproj[cs, :]),
        })
    bkr = run_bass_kernel_spmd(nc, in_maps, list(range(NCORES)), **spmd_kwargs)
    res = bkr.results

    attn = np.empty((B, H, N, NY), dtype=np.float32)
    out = np.zeros((B, N, C), dtype=np.float32)
    for c in range(NCORES):
        b, hg = divmod(c, NCORES // B)
        attn[b, hg * HL:(hg + 1) * HL] = res[c]["attn_o"].transpose(0, 2, 1)
        out[b] += res[c]["out_o"]
    out += bproj[None, None, :]
    return out, attn, bkr


def kernel(x, y, Wq, Wkv, Wproj, bproj):
    out, attn, _ = run(x, y, Wq, Wkv, Wproj, bproj)
    return out, attn
